# revision 10
# baseline (speedup 1.0000x reference)
"""GRU (5-layer, H=128) Trainium2 Bass kernel.

Strategy: pure data parallel over batch (64 / 8 cores = 8 per core).
Per core, the 5 layers run as a chunk-staggered wavefront (chunk C=16
timesteps): layer l processes chunk (m - l) during "round" m.  All
per-step elementwise work for the 5 active layers is batched into one
7-op chain (sigmoid / mul / add / tanh / sub / mul / add) operating on
[128, nl*8] tiles.  Gate pre-activations are accumulated directly in
PSUM by the tensor engine: per-chunk bias matmuls (rank<=4 ones-trick),
per-chunk input-gate matmuls (amortized weight loads), and per-step
hidden-gate matmuls.

Layouts (per core, SBUF):
  hW  [128, 5, 577, 8]  h history; slot W stores h_l(t) at W = t + 16*l + 1
                        (W = 16l holds the per-layer zero initial state)
  xT  [128, 4, T, 8]    transposed input (built on-chip via PE transpose)
  P_all (PSUM) [128, 3, 5, 16, 8]  r/z/xn pre-activations for one chunk
  P_hn  (PSUM) [128, 5, 16, 8]     W_hn h + b_hhn for one chunk
"""

import sys

for p in ("/opt/trn_rl_repo", "/opt/pypackages"):
    if p not in sys.path:
        sys.path.append(p)

import numpy as np
import ml_dtypes

BFNP = ml_dtypes.bfloat16

import concourse.bass as bass  # noqa: F401
import concourse.mybir as mybir
import concourse.tile as tile
from concourse import bacc
from concourse.masks import make_identity

F32 = mybir.dt.float32
BF16 = mybir.dt.bfloat16
AF = mybir.ActivationFunctionType
ALU = mybir.AluOpType

H = 128
L = 5
NCORE = 8
BC = 8  # batch per core
IN = 512
OUT = 96
C = 16  # chunk (timesteps)


DEBUG = False


def build_nc(T=512):
    NCH = T // C
    NR = NCH + L - 1
    WDIM = T + C * (L - 1) + 1

    nc = bacc.Bacc("TRN2", target_bir_lowering=False, debug=False)

    xp = nc.dram_tensor("xp", [T, BC, IN], BF16, kind="ExternalInput")
    whhT = nc.dram_tensor("whhT", [H, L, 3, H], BF16, kind="ExternalInput")
    wihT = nc.dram_tensor("wihT", [H, L - 1, 3, H], BF16, kind="ExternalInput")
    wih0T = nc.dram_tensor("wih0T", [H, 4, 3, H], BF16, kind="ExternalInput")
    fcT = nc.dram_tensor("fcT", [H, OUT], BF16, kind="ExternalInput")
    biasP = nc.dram_tensor("biasP", [4, 4, H], F32, kind="ExternalInput")
    oh4 = nc.dram_tensor("oh4", [4, 512], F32, kind="ExternalInput")
    bhn = nc.dram_tensor("bhn", [4, H], F32, kind="ExternalInput")
    bl4 = nc.dram_tensor("bl4", [1, H], F32, kind="ExternalInput")
    on1 = nc.dram_tensor("on1", [1, H], F32, kind="ExternalInput")
    fcb = nc.dram_tensor("fcb", [BC, OUT], F32, kind="ExternalInput")
    y = nc.dram_tensor("y", [BC, OUT], F32, kind="ExternalOutput")
    dbg_h = nc.dram_tensor("dbg_h", [H, L, T + 1, BC], BF16, kind="ExternalOutput") if DEBUG else None
    dbg_x = nc.dram_tensor("dbg_x", [H, 4, T, BC], BF16, kind="ExternalOutput") if DEBUG else None
    dbg_p = nc.dram_tensor("dbg_p", [H, 3, C, BC], F32, kind="ExternalOutput") if DEBUG else None
    dbg_pn = nc.dram_tensor("dbg_pn", [H, C, BC], F32, kind="ExternalOutput") if DEBUG else None

    with tile.TileContext(nc) as tc:
        with (
            tc.tile_pool(name="persist", bufs=1) as pp,
            tc.tile_pool(name="xsrc", bufs=3) as xsp,
            tc.tile_pool(name="tmp", bufs=3) as tp,
            tc.tile_pool(name="pall", bufs=1, space="PSUM") as pallp,
            tc.tile_pool(name="phn", bufs=1, space="PSUM") as phnp,
            tc.tile_pool(name="ptp", bufs=1, space="PSUM") as ptpp,
            tc.tile_pool(name="pfc", bufs=1, space="PSUM") as pfcp,
        ):
            hW = pp.tile([H, L, WDIM, BC], BF16, tag="hW")
            xT = pp.tile([H, 4, T, BC], BF16, tag="xT")
            whh_sb = pp.tile([H, L, 3, H], BF16, tag="whh")
            wih_sb = pp.tile([H, L - 1, 3, H], BF16, tag="wih")
            wih0_sb = pp.tile([H, 4, 3, H], BF16, tag="wih0")
            fcT_sb = pp.tile([H, OUT], BF16, tag="fcT")
            biasP_sb = pp.tile([4, 4, H], F32, tag="biasP")
            oh4_sb = pp.tile([4, 512], F32, tag="oh4")
            bhn_sb = pp.tile([4, H], F32, tag="bhn")
            bl4_sb = pp.tile([1, H], F32, tag="bl4")
            on1_sb = pp.tile([1, H], F32, tag="on1")
            fcb_sb = pp.tile([BC, OUT], F32, tag="fcb")
            ident = pp.tile([H, H], BF16, tag="ident")

            nc.sync.dma_start(whh_sb[:, :, :, :], whhT[:, :, :, :])
            nc.sync.dma_start(wih_sb[:, :, :, :], wihT[:, :, :, :])
            nc.sync.dma_start(wih0_sb[:, :, :, :], wih0T[:, :, :, :])
            nc.sync.dma_start(fcT_sb[:, :], fcT[:, :])
            nc.sync.dma_start(biasP_sb[:, :, :], biasP[:, :, :])
            nc.sync.dma_start(oh4_sb[:, :], oh4[:, :])
            nc.sync.dma_start(bhn_sb[:, :], bhn[:, :])
            nc.sync.dma_start(bl4_sb[:, :], bl4[:, :])
            nc.sync.dma_start(on1_sb[:, :], on1[:, :])
            nc.sync.dma_start(fcb_sb[:, :], fcb[:, :])
            make_identity(nc, ident[:, :])

            for l in range(L):
                nc.vector.memset(hW[:, l, C * l, :], 0.0)

            def load_x_chunk(m):
                xs = xsp.tile([C * BC, IN], BF16, tag="xs")
                nc.sync.dma_start(xs[:, :], xp[m * C : (m + 1) * C, :, :])
                for ki in range(4):
                    tpp = ptpp.tile([H, C * BC], BF16, tag="tp")
                    nc.tensor.transpose(
                        tpp[:, :], xs[:, ki * H : (ki + 1) * H], ident[:, :]
                    )
                    nc.scalar.copy(xT[:, ki, m * C : (m + 1) * C, :], tpp[:, :])

            load_x_chunk(0)

            for m in range(NR):
                la0 = max(0, m - (NCH - 1))
                la1 = min(L - 1, m)
                sl = slice(la0, la1 + 1)

                P_all = pallp.tile([H, 3, L, C, BC], F32, tag="P_all")
                P_hn = phnp.tile([H, L, C, BC], F32, tag="P_hn")
                Pfl = P_all[:, :, :, :, :].rearrange("p g l c b -> p (g l c b)")
                Phfl = P_hn[:, :, :, :].rearrange("p l c b -> p (l c b)")

                if m + 1 < NCH:
                    load_x_chunk(m + 1)

                # bias accumulation (start=True) via rank<=4 ones-trick
                for bk in range(4):
                    kk = 4 if bk < 3 else 3
                    N = 512 if bk < 3 else 384
                    nc.tensor.matmul(
                        Pfl[:, bk * 512 : bk * 512 + N],
                        biasP_sb[0:kk, bk, :],
                        oh4_sb[0:kk, 0:N],
                        start=True,
                        stop=False,
                        skip_group_check=True,
                    )
                nc.tensor.matmul(
                    Phfl[:, 0:512],
                    bhn_sb[0:4, :],
                    oh4_sb[0:4, 0:512],
                    start=True,
                    stop=False,
                    skip_group_check=True,
                )
                nc.tensor.matmul(
                    Phfl[:, 512:640],
                    bl4_sb[0:1, :],
                    on1_sb[0:1, 0:H],
                    start=True,
                    stop=False,
                    skip_group_check=True,
                )

                # input-gate (ih) chunk matmuls
                if m < NCH:  # layer 0 reads xT chunk m
                    for g in range(3):
                        for ki in range(4):
                            nc.tensor.matmul(
                                P_all[:, g, 0, :, :],
                                wih0_sb[:, ki, g, :],
                                xT[:, ki, m * C : (m + 1) * C, :],
                                start=False,
                                stop=False,
                                skip_group_check=True,
                            )
                for l in range(max(1, la0), la1 + 1):
                    for g in range(3):
                        nc.tensor.matmul(
                            P_all[:, g, l, :, :],
                            wih_sb[:, l - 1, g, :],
                            hW[:, l - 1, C * m - 15 : C * m + 1, :],
                            start=False,
                            stop=False,
                            skip_group_check=True,
                        )

                if DEBUG and m == 1:
                    dbp = pp.tile([H, 3, C, BC], F32, tag="dbp")
                    dbpn = pp.tile([H, C, BC], F32, tag="dbpn")
                    nc.scalar.copy(dbp[:, :, :, :], P_all[:, :, 1, :, :])
                    nc.scalar.copy(dbpn[:, :, :], P_hn[:, 1, :, :])
                    nc.sync.dma_start(dbg_p[:, :, :, :], dbp[:, :, :, :])
                    nc.sync.dma_start(dbg_pn[:, :, :], dbpn[:, :, :])

                for j in range(C):
                    base = C * m + j
                    # hidden-gate matmuls for this step
                    for l in range(la0, la1 + 1):
                        for g in range(3):
                            dest = (
                                P_all[:, g, l, j, :]
                                if g < 2
                                else P_hn[:, l, j, :]
                            )
                            nc.tensor.matmul(
                                dest,
                                whh_sb[:, l, g, :],
                                hW[:, l, base, :],
                                start=False,
                                stop=True,
                                skip_group_check=True,
                            )
                    # elementwise chain, batched over active layers
                    rzt = tp.tile([H, 2, L, BC], F32, tag="rz")
                    rnt = tp.tile([H, L, BC], F32, tag="rn")
                    npret = tp.tile([H, L, BC], F32, tag="npre")
                    nt = tp.tile([H, L, BC], F32, tag="nt")
                    dt_ = tp.tile([H, L, BC], F32, tag="dt")
                    zdt = tp.tile([H, L, BC], F32, tag="zd")
                    nc.scalar.activation(
                        rzt[:, :, sl, :], P_all[:, 0:2, sl, j, :], AF.Sigmoid
                    )
                    nc.vector.tensor_tensor(
                        rnt[:, sl, :], rzt[:, 0, sl, :], P_hn[:, sl, j, :], ALU.mult
                    )
                    nc.vector.tensor_tensor(
                        npret[:, sl, :], rnt[:, sl, :], P_all[:, 2, sl, j, :], ALU.add
                    )
                    nc.scalar.activation(nt[:, sl, :], npret[:, sl, :], AF.Tanh)
                    nc.vector.tensor_tensor(
                        dt_[:, sl, :], hW[:, sl, base, :], nt[:, sl, :], ALU.subtract
                    )
                    nc.vector.tensor_tensor(
                        zdt[:, sl, :], rzt[:, 1, sl, :], dt_[:, sl, :], ALU.mult
                    )
                    nc.vector.tensor_tensor(
                        hW[:, sl, base + 1, :], nt[:, sl, :], zdt[:, sl, :], ALU.add
                    )

            # final FC on last timestep of layer 4
            pfc = pfcp.tile([BC, OUT], F32, tag="fc")
            nc.tensor.matmul(
                pfc[:, :],
                hW[:, L - 1, WDIM - 1, :],
                fcT_sb[:, :],
                start=True,
                stop=True,
                skip_group_check=True,
            )
            out_sb = pp.tile([BC, OUT], F32, tag="out")
            nc.vector.tensor_tensor(out_sb[:, :], pfc[:, :], fcb_sb[:, :], ALU.add)
            nc.sync.dma_start(y[:, :], out_sb[:, :])
            if DEBUG:
                for l in range(L):
                    nc.sync.dma_start(
                        dbg_h[:, l, :, :], hW[:, l, C * l : C * l + T + 1, :]
                    )
                nc.sync.dma_start(dbg_x[:, :, :, :], xT[:, :, :, :])

    nc.compile()
    return nc


def prep_shared(w_ih0, w_ih_rest, w_hh, b_ih, b_hh, fc_w, fc_b):
    d = {}
    whhT = np.empty([H, L, 3, H], np.float32)
    for l in range(L):
        for g in range(3):
            whhT[:, l, g, :] = w_hh[l, g * H : (g + 1) * H, :].T
    d["whhT"] = whhT.astype(BFNP)
    wihT = np.empty([H, L - 1, 3, H], np.float32)
    for l in range(1, L):
        for g in range(3):
            wihT[:, l - 1, g, :] = w_ih_rest[l - 1, g * H : (g + 1) * H, :].T
    d["wihT"] = wihT.astype(BFNP)
    wih0T = np.empty([H, 4, 3, H], np.float32)
    for ki in range(4):
        for g in range(3):
            wih0T[:, ki, g, :] = w_ih0[g * H : (g + 1) * H, ki * H : (ki + 1) * H].T
    d["wih0T"] = wih0T.astype(BFNP)
    d["fcT"] = np.ascontiguousarray(fc_w.T).astype(BFNP)
    biasP = np.zeros([4, 4, H], np.float32)
    for i in range(15):  # chunk index i = g*5 + l
        g, l = divmod(i, 5)
        b = b_ih[l, g * H : (g + 1) * H].astype(np.float32)
        if g < 2:
            b = b + b_hh[l, g * H : (g + 1) * H]
        biasP[i % 4, i // 4, :] = b  # dims: [k, bank, H]
    d["biasP"] = biasP
    oh4 = np.zeros([4, 512], np.float32)
    for k in range(4):
        oh4[k, k * H : (k + 1) * H] = 1.0
    d["oh4"] = oh4
    d["bhn"] = np.ascontiguousarray(b_hh[0:4, 2 * H : 3 * H].astype(np.float32))
    d["bl4"] = np.ascontiguousarray(b_hh[4:5, 2 * H : 3 * H].astype(np.float32))
    d["on1"] = np.ones([1, H], np.float32)
    d["fcb"] = np.tile(fc_b.astype(np.float32)[None, :], (BC, 1))
    return d


_NC_CACHE = {}


def run(x, w_ih0, w_ih_rest, w_hh, b_ih, b_hh, fc_w, fc_b, T=512, **run_kwargs):
    from concourse.bass_utils import run_bass_kernel_spmd

    if T not in _NC_CACHE:
        _NC_CACHE[T] = build_nc(T)
    nc = _NC_CACHE[T]
    shared = prep_shared(
        np.asarray(w_ih0), np.asarray(w_ih_rest), np.asarray(w_hh),
        np.asarray(b_ih), np.asarray(b_hh), np.asarray(fc_w), np.asarray(fc_b),
    )
    x = np.asarray(x)
    in_maps = []
    for c in range(NCORE):
        m = dict(shared)
        xs = x[c * BC : (c + 1) * BC, :T, :]  # [BC, T, IN]
        m["xp"] = np.ascontiguousarray(xs.transpose(1, 0, 2)).astype(BFNP)
        in_maps.append(m)
    res = run_bass_kernel_spmd(nc, in_maps, core_ids=list(range(NCORE)), **run_kwargs)
    out = np.concatenate([res.results[c]["y"] for c in range(NCORE)], axis=0)
    return out, res


def kernel(x, w_ih0, w_ih_rest, w_hh, b_ih, b_hh, fc_w, fc_b):
    out, _ = run(x, w_ih0, w_ih_rest, w_hh, b_ih, b_hh, fc_w, fc_b, T=512)
    return out.astype(np.float32)


if __name__ == "__main__":
    # quick smoke test at small T against a numpy reference
    T = int(sys.argv[1]) if len(sys.argv) > 1 else 64
    rng = np.random.default_rng(0)
    s = 1.0 / np.sqrt(H)
    u = lambda *sh: rng.uniform(-s, s, sh).astype(np.float32)
    x = rng.standard_normal((64, T, IN), dtype=np.float32)
    w_ih0 = u(3 * H, IN)
    w_ih_rest = u(L - 1, 3 * H, H)
    w_hh = u(L, 3 * H, H)
    b_ih = u(L, 3 * H)
    b_hh = u(L, 3 * H)
    fc_w = u(OUT, H)
    fc_b = u(OUT)

    def np_ref():
        sig = lambda v: 1.0 / (1.0 + np.exp(-v))
        h_in = x.astype(np.float64)
        for l in range(L):
            wi = (w_ih0 if l == 0 else w_ih_rest[l - 1]).astype(np.float64)
            wh = w_hh[l].astype(np.float64)
            gx = np.einsum("bti,gi->btg", h_in, wi) + b_ih[l]
            h = np.zeros((64, H))
            hs = []
            for t in range(T):
                gh = h @ wh.T + b_hh[l]
                xr, xz, xn = np.split(gx[:, t], 3, -1)
                hr, hz, hn = np.split(gh, 3, -1)
                r = sig(xr + hr)
                z = sig(xz + hz)
                n = np.tanh(xn + r * hn)
                h = (1 - z) * n + z * h
                hs.append(h)
            h_in = np.stack(hs, 1)
        return h_in[:, -1] @ fc_w.astype(np.float64).T + fc_b

    exp = np_ref()
    got, res = run(x, w_ih0, w_ih_rest, w_hh, b_ih, b_hh, fc_w, fc_b, T=T)
    err = np.abs(got - exp)
    rel = np.linalg.norm(got - exp) / np.linalg.norm(exp)
    print("max abs err:", err.max(), "rel:", rel)
    print("exec_time_ns:", res.exec_time_ns)
